# revision 5
# baseline (speedup 1.0000x reference)
"""Distributed Trainium2 kernel for nn_Attention_2654289789382 (sparse_attention).

Math (reference):
    sigma = sigmoid(x @ W_sigma + b_sigma)           (b, h, n)
    den_i = exp(sigma)+1 ;  r_i = 1/den_i = sigmoid(-sigma)   in (0.2689, 0.5)
    prior[i,j] = softmax_j(-|i-j| * r_i)
    out = (prior @ v) reshaped @ W_out + b_out,  v = x @ W_v

Key structure exploited:
  * r_i >= 0.2689  =>  prior terms with |i-j| > 128 are < 1.1e-15 relative:
    the attention matrix is effectively banded (+-128).  Per 128-row i-block
    only 3 j-tiles of 128 contribute.
  * softmax denominator has a closed form (two-sided geometric series):
        Z_i = 1 + (2z - z^(i+1) - z^(n-i)) / (1-z),  z = exp(-r_i)
  * normalized P tile [i_part, j_free] = Exp(dist * (-r_i) - ln(Z_i)) is ONE
    ScalarE activation op (per-partition scale & bias), then PE-transposed to
    Q[j, i] for the AV matmul.  out^T[dh, i] lands in exactly the layout the
    W_out projection needs as lhsT.

Sharding: 8 cores = 4 batches x 2 sequence halves. No collectives; each core
computes final output rows for its (batch, half) independently.
"""

import numpy as np

import concourse.bass as bass
import concourse.mybir as mybir
import concourse.tile as tile
from concourse import bacc
from concourse.bass_utils import run_bass_kernel_spmd

F32 = mybir.dt.float32

B, N, D = 4, 2048, 512
H, DH = 8, 64
HALF = N // 2            # 1024 rows per core
PAD = 128                # band halo
NJROWS = HALF + 2 * PAD  # 1280 padded j rows per core
NBLK = HALF // 128       # 8 i-blocks per core
NJT = NJROWS // 128      # 10 j-tiles per core

_nc_cache = None


def _build_nc():
    nc = bacc.Bacc("TRN2", target_bir_lowering=False, debug=False)

    # ---- DRAM parameters (per-core shard contents supplied via in_maps) ----
    xT = nc.dram_tensor("xT", [D, NJROWS], F32, kind="ExternalInput")
    Wv = nc.dram_tensor("Wv", [D, D], F32, kind="ExternalInput")
    Wsig = nc.dram_tensor("Wsig", [D, H], F32, kind="ExternalInput")
    Wo = nc.dram_tensor("Wo", [D, D], F32, kind="ExternalInput")
    bsig = nc.dram_tensor("bsig", [128, H], F32, kind="ExternalInput")
    bout = nc.dram_tensor("bout", [128, D], F32, kind="ExternalInput")
    mdist = nc.dram_tensor("mdist", [128, 384], F32, kind="ExternalInput")
    ident = nc.dram_tensor("ident", [128, 128], F32, kind="ExternalInput")
    ivp1 = nc.dram_tensor("ivp1", [128, NBLK * H], F32, kind="ExternalInput")
    ivnm = nc.dram_tensor("ivnm", [128, NBLK * H], F32, kind="ExternalInput")
    out = nc.dram_tensor("out", [HALF, D], F32, kind="ExternalOutput")

    EXP = mybir.ActivationFunctionType.Exp
    SIGM = mybir.ActivationFunctionType.Sigmoid
    LN = mybir.ActivationFunctionType.Ln
    MUL = mybir.AluOpType.mult
    ADD = mybir.AluOpType.add

    with tile.TileContext(nc) as tc:
        with (
            tc.tile_pool(name="const", bufs=1) as cpool,
            tc.tile_pool(name="vpool", bufs=1) as vpool,
            tc.tile_pool(name="otpool", bufs=1) as otpool,
            tc.tile_pool(name="sg", bufs=1) as sgpool,
        ):
            # ---------------- loads ----------------
            xT_t = []
            for dt in range(4):
                t = cpool.tile([128, NJROWS], F32, tag=f"xT{dt}")
                nc.sync.dma_start(t[:], xT[dt * 128:(dt + 1) * 128, :])
                xT_t.append(t)
            Wv_t, Wo_t, Wsig_t = [], [], []
            for dt in range(4):
                t = cpool.tile([128, D], F32, tag=f"Wv{dt}")
                nc.sync.dma_start(t[:], Wv[dt * 128:(dt + 1) * 128, :])
                Wv_t.append(t)
            for dt in range(4):
                t = cpool.tile([128, D], F32, tag=f"Wo{dt}")
                nc.sync.dma_start(t[:], Wo[dt * 128:(dt + 1) * 128, :])
                Wo_t.append(t)
            for dt in range(4):
                t = cpool.tile([128, H], F32, tag=f"Wsig{dt}")
                nc.sync.dma_start(t[:], Wsig[dt * 128:(dt + 1) * 128, :])
                Wsig_t.append(t)
            bsig_t = cpool.tile([128, H], F32, tag="bsig")
            nc.sync.dma_start(bsig_t[:], bsig[:, :])
            bout_t = cpool.tile([128, D], F32, tag="bout")
            nc.sync.dma_start(bout_t[:], bout[:, :])
            m_t = cpool.tile([128, 384], F32, tag="mdist")
            nc.sync.dma_start(m_t[:], mdist[:, :])
            id_t = cpool.tile([128, 128], F32, tag="ident")
            nc.sync.dma_start(id_t[:], ident[:, :])
            ivp1_t = cpool.tile([128, NBLK * H], F32, tag="ivp1")
            nc.sync.dma_start(ivp1_t[:], ivp1[:, :])
            ivnm_t = cpool.tile([128, NBLK * H], F32, tag="ivnm")
            nc.sync.dma_start(ivnm_t[:], ivnm[:, :])

            # ---------------- prologue: V = x @ W_v  (all heads, padded j) --
            V_t = []
            with tc.tile_pool(name="psv", bufs=2, space="PSUM") as psv:
                for jt in range(NJT):
                    pv = psv.tile([128, D], F32, tag="pv")
                    for dt in range(4):
                        nc.tensor.matmul(
                            pv[:],
                            lhsT=xT_t[dt][:, jt * 128:(jt + 1) * 128],
                            rhs=Wv_t[dt][:],
                            start=(dt == 0),
                            stop=(dt == 3),
                        )
                    vt = vpool.tile([128, D], F32, tag=f"V{jt}")
                    nc.scalar.copy(vt[:], pv[:])
                    V_t.append(vt)

                # ------------ sigma / closed-form log-denominator ----------
                # s_all[p, b*8+h] = logits for i-block b, head h
                s_all = sgpool.tile([128, NBLK * H], F32, tag="s_all")
                for b in range(NBLK):
                    ps = psv.tile([128, H], F32, tag="ps")
                    for dt in range(4):
                        nc.tensor.matmul(
                            ps[:],
                            lhsT=xT_t[dt][:, PAD + b * 128:PAD + (b + 1) * 128],
                            rhs=Wsig_t[dt][:],
                            start=(dt == 0),
                            stop=(dt == 3),
                        )
                    nc.vector.tensor_add(
                        s_all[:, b * H:(b + 1) * H], ps[:], bsig_t[:]
                    )

            sig = sgpool.tile([128, NBLK * H], F32, tag="sig")
            nc.scalar.activation(sig[:], s_all[:], SIGM)
            r_all = sgpool.tile([128, NBLK * H], F32, tag="r_all")
            nc.scalar.activation(r_all[:], sig[:], SIGM, scale=-1.0)
            negr = sgpool.tile([128, NBLK * H], F32, tag="negr")
            nc.vector.tensor_scalar_mul(negr[:], r_all[:], -1.0)
            z = sgpool.tile([128, NBLK * H], F32, tag="z")
            nc.scalar.activation(z[:], negr[:], EXP)
            argA = sgpool.tile([128, NBLK * H], F32, tag="argA")
            nc.vector.tensor_mul(argA[:], negr[:], ivp1_t[:])
            expA = sgpool.tile([128, NBLK * H], F32, tag="expA")
            nc.scalar.activation(expA[:], argA[:], EXP)
            argB = sgpool.tile([128, NBLK * H], F32, tag="argB")
            nc.vector.tensor_mul(argB[:], negr[:], ivnm_t[:])
            expB = sgpool.tile([128, NBLK * H], F32, tag="expB")
            nc.scalar.activation(expB[:], argB[:], EXP)
            w = sgpool.tile([128, NBLK * H], F32, tag="w")
            nc.vector.tensor_scalar(w[:], z[:], -1.0, 1.0, MUL, ADD)
            rw = sgpool.tile([128, NBLK * H], F32, tag="rw")
            nc.vector.reciprocal(rw[:], w[:])
            t1 = sgpool.tile([128, NBLK * H], F32, tag="t1")
            nc.vector.tensor_scalar_mul(t1[:], z[:], 2.0)
            nc.vector.tensor_sub(t1[:], t1[:], expA[:])
            nc.vector.tensor_sub(t1[:], t1[:], expB[:])
            t2 = sgpool.tile([128, NBLK * H], F32, tag="t2")
            nc.vector.tensor_mul(t2[:], t1[:], rw[:])
            lnz = sgpool.tile([128, NBLK * H], F32, tag="lnz")
            nc.scalar.activation(lnz[:], t2[:], LN, bias=1.0)
            negln = sgpool.tile([128, NBLK * H], F32, tag="negln")
            nc.vector.tensor_scalar_mul(negln[:], lnz[:], -1.0)

            # persistent out^T tiles: tile t holds heads 2t,2t+1 -> 128 rows
            outT_t = []
            for t in range(4):
                oT = otpool.tile([128, HALF], F32, tag=f"oT{t}")
                outT_t.append(oT)

            # ---------------- main loop ----------------
            with (
                tc.tile_pool(name="pp", bufs=3) as ppool,
                tc.tile_pool(name="qq", bufs=6) as qpool,
                tc.tile_pool(name="fin", bufs=3) as fpool,
                tc.tile_pool(name="psq", bufs=3, space="PSUM") as psq,
                tc.tile_pool(name="psa", bufs=2, space="PSUM") as psa,
                tc.tile_pool(name="psf", bufs=2, space="PSUM") as psf,
            ):
                for b in range(NBLK):
                    for h in range(H):
                        c = b * H + h
                        P = ppool.tile([128, 384], F32, tag="P")
                        nc.scalar.activation(
                            P[:], m_t[:],
                            EXP,
                            bias=negln[:, c:c + 1],
                            scale=negr[:, c:c + 1],
                        )
                        pav = psa.tile([64, 128], F32, tag="pav")
                        for o in range(3):
                            pq = psq.tile([128, 128], F32, tag="pq")
                            nc.tensor.transpose(
                                pq[:], P[:, o * 128:(o + 1) * 128], id_t[:]
                            )
                            qs = qpool.tile([128, 128], F32, tag="qs")
                            nc.vector.tensor_copy(qs[:], pq[:])
                            nc.tensor.matmul(
                                pav[:],
                                lhsT=V_t[b + o][:, h * 64:(h + 1) * 64],
                                rhs=qs[:],
                                start=(o == 0),
                                stop=(o == 2),
                            )
                        nc.scalar.copy(
                            outT_t[h // 2][
                                (h % 2) * 64:(h % 2) * 64 + 64,
                                b * 128:(b + 1) * 128,
                            ],
                            pav[:],
                        )
                    # ---- projection for i-block b ----
                    pf = psf.tile([128, D], F32, tag="pf")
                    for t in range(4):
                        nc.tensor.matmul(
                            pf[:],
                            lhsT=outT_t[t][:, b * 128:(b + 1) * 128],
                            rhs=Wo_t[t][:],
                            start=(t == 0),
                            stop=(t == 3),
                        )
                    fin = fpool.tile([128, D], F32, tag="fin")
                    nc.vector.tensor_add(fin[:], pf[:], bout_t[:])
                    nc.sync.dma_start(out[b * 128:(b + 1) * 128, :], fin[:])

    nc.compile()
    return nc


def _make_in_maps(x, W_v, W_sigma, b_sigma, W_out, b_out):
    mdist = np.abs(
        np.arange(384, dtype=np.float32)[None, :]
        - 128.0
        - np.arange(128, dtype=np.float32)[:, None]
    ).astype(np.float32)
    ident = np.eye(128, dtype=np.float32)
    bsig_b = np.broadcast_to(b_sigma[None, :], (128, H)).copy().astype(np.float32)
    bout_b = np.broadcast_to(b_out[None, :], (128, D)).copy().astype(np.float32)

    in_maps = []
    for c in range(8):
        bb, half = c // 2, c % 2
        i_start = half * HALF
        # padded j rows: j_abs in [i_start-128, i_start+HALF+128)
        xp = np.zeros((NJROWS, D), dtype=np.float32)
        j_lo = max(0, i_start - PAD)
        j_hi = min(N, i_start + HALF + PAD)
        xp[j_lo - (i_start - PAD):j_hi - (i_start - PAD)] = x[bb, j_lo:j_hi]
        xT = np.ascontiguousarray(xp.T)

        # i_abs per (partition p, block b): i_start + b*128 + p
        p = np.arange(128, dtype=np.float32)[:, None]
        blk = np.arange(NBLK, dtype=np.float32)[None, :]
        i_abs = i_start + blk * 128 + p  # [128, NBLK]
        ivp1 = np.repeat(i_abs + 1.0, H, axis=1).astype(np.float32)
        ivnm = np.repeat(float(N) - i_abs, H, axis=1).astype(np.float32)

        in_maps.append(
            {
                "xT": xT,
                "Wv": np.ascontiguousarray(W_v, dtype=np.float32),
                "Wsig": np.ascontiguousarray(W_sigma, dtype=np.float32),
                "Wo": np.ascontiguousarray(W_out, dtype=np.float32),
                "bsig": bsig_b,
                "bout": bout_b,
                "mdist": mdist,
                "ident": ident,
                "ivp1": ivp1,
                "ivnm": ivnm,
            }
        )
    return in_maps


def kernel(x, W_v, W_sigma, b_sigma, W_out, b_out):
    global _nc_cache
    x = np.asarray(x, dtype=np.float32)
    W_v = np.asarray(W_v, dtype=np.float32)
    W_sigma = np.asarray(W_sigma, dtype=np.float32)
    b_sigma = np.asarray(b_sigma, dtype=np.float32)
    W_out = np.asarray(W_out, dtype=np.float32)
    b_out = np.asarray(b_out, dtype=np.float32)

    if _nc_cache is None:
        _nc_cache = _build_nc()
    nc = _nc_cache

    in_maps = _make_in_maps(x, W_v, W_sigma, b_sigma, W_out, b_out)
    res = run_bass_kernel_spmd(nc, in_maps, core_ids=list(range(8)))

    out = np.empty((B, N, D), dtype=np.float32)
    for c in range(8):
        bb, half = c // 2, c % 2
        out[bb, half * HALF:(half + 1) * HALF, :] = res.results[c]["out"]
    return out


# revision 9
# speedup vs baseline: 1.7098x; 1.7098x over previous
"""Distributed Trainium2 kernel for nn_Attention_2654289789382 (sparse_attention).

Math (reference):
    sigma = sigmoid(x @ W_sigma + b_sigma)           (b, h, n)
    den_i = exp(sigma)+1 ;  r_i = 1/den_i = sigmoid(-sigma)   in (0.2689, 0.5)
    prior[i,j] = softmax_j(-|i-j| * r_i)
    out = (prior @ v) reshaped @ W_out + b_out,  v = x @ W_v

Structure exploited:
  * r_i >= 0.2689  =>  prior terms with |i-j| > 128 are < 1.1e-15 relative:
    the attention matrix is banded (+-128); per 128-row i-block only 3
    j-tiles of 128 contribute.
  * softmax denominator in closed form (two-sided geometric series):
        den_i = 1 + (2z - z^(i+1) - z^(n-i)) / (1-z),  z = exp(-r_i)
  * Q[j,i] = exp(|i-j| * -r_i) built directly in matmul-rhs layout:
    -r broadcast along partitions via DMA (idle engines), one DVE multiply
    with a precomputed |dist| master, one ScalarE Exp -> bf16.
  * AV matmul in bf16 -> out^T[dh,i] psum; normalization (1/den, also
    DMA-broadcast) fused into the psum->SBUF copy (DVE multiply, bf16 out).
    out^T lands in exactly the lhsT layout the W_out projection needs.

Sharding: 8 cores = 4 batches x 2 sequence halves; no collectives.
"""

import numpy as np
import ml_dtypes

import concourse.bass as bass
import concourse.mybir as mybir
import concourse.tile as tile
from concourse import bacc
from concourse.bass_utils import run_bass_kernel_spmd

F32 = mybir.dt.float32
BF16 = mybir.dt.bfloat16

B, N, D = 4, 2048, 512
H, DH = 8, 64
HALF = N // 2            # 1024 rows per core
PAD = 128                # band halo
NJROWS = HALF + 2 * PAD  # 1280 padded j rows per core
NBLK = HALF // 128       # 8 i-blocks per core
NJT = NJROWS // 128      # 10 j-tiles per core

_nc_cache = None


def _build_nc():
    nc = bacc.Bacc("TRN2", target_bir_lowering=False, debug=False)

    xTb = nc.dram_tensor("xTb", [D, NJROWS], BF16, kind="ExternalInput")
    Wvb = nc.dram_tensor("Wvb", [D, D], BF16, kind="ExternalInput")
    Wsb = nc.dram_tensor("Wsb", [D, H], BF16, kind="ExternalInput")
    Wob = nc.dram_tensor("Wob", [D, D], BF16, kind="ExternalInput")
    bsig = nc.dram_tensor("bsig", [H, 1], F32, kind="ExternalInput")
    bout = nc.dram_tensor("bout", [128, D], F32, kind="ExternalInput")
    m2r = nc.dram_tensor("m2r", [128, 384], F32, kind="ExternalInput")
    ivp1 = nc.dram_tensor("ivp1", [H, HALF], F32, kind="ExternalInput")
    ivnm = nc.dram_tensor("ivnm", [H, HALF], F32, kind="ExternalInput")
    out = nc.dram_tensor("out", [HALF, D], F32, kind="ExternalOutput")
    negr_d = nc.dram_tensor("negr_d", [H, HALF], F32)
    inv_d = nc.dram_tensor("inv_d", [H, HALF], F32)

    EXP = mybir.ActivationFunctionType.Exp
    SIGM = mybir.ActivationFunctionType.Sigmoid
    MUL = mybir.AluOpType.mult
    ADD = mybir.AluOpType.add

    with tile.TileContext(nc) as tc:
        with (
            tc.tile_pool(name="const", bufs=1) as cpool,
            tc.tile_pool(name="vpool", bufs=1) as vpool,
            tc.tile_pool(name="otpool", bufs=1) as otpool,
            tc.tile_pool(name="sg", bufs=1) as sgpool,
            tc.tile_pool(name="bc", bufs=1) as bcpool,
        ):
            # ---------------- loads ----------------
            xTb_t = []
            for dt in range(4):
                t = cpool.tile([128, NJROWS], BF16, tag=f"xTb{dt}")
                nc.sync.dma_start(t[:], xTb[dt * 128:(dt + 1) * 128, :])
                xTb_t.append(t)
            Wvb_t, Wob_t, Wsb_t = [], [], []
            for dt in range(4):
                t = cpool.tile([128, D], BF16, tag=f"Wvb{dt}")
                nc.sync.dma_start(t[:], Wvb[dt * 128:(dt + 1) * 128, :])
                Wvb_t.append(t)
            for dt in range(4):
                t = cpool.tile([128, D], BF16, tag=f"Wob{dt}")
                nc.sync.dma_start(t[:], Wob[dt * 128:(dt + 1) * 128, :])
                Wob_t.append(t)
            for dt in range(4):
                t = cpool.tile([128, H], BF16, tag=f"Wsb{dt}")
                nc.sync.dma_start(t[:], Wsb[dt * 128:(dt + 1) * 128, :])
                Wsb_t.append(t)
            bsig_t = cpool.tile([H, 1], F32, tag="bsig")
            nc.sync.dma_start(bsig_t[:], bsig[:, :])
            bout_t = cpool.tile([128, D], F32, tag="bout")
            nc.sync.dma_start(bout_t[:], bout[:, :])
            m2r_t = cpool.tile([128, 384], F32, tag="m2r")
            nc.sync.dma_start(m2r_t[:], m2r[:, :])
            ivp1_t = cpool.tile([H, HALF], F32, tag="ivp1")
            nc.sync.dma_start(ivp1_t[:], ivp1[:, :])
            ivnm_t = cpool.tile([H, HALF], F32, tag="ivnm")
            nc.sync.dma_start(ivnm_t[:], ivnm[:, :])

            # ---------------- prologue ----------------
            # V = x @ W_v  (bf16, all heads, padded j rows)
            V_t = []
            with (
                tc.tile_pool(name="psv", bufs=2, space="PSUM") as psv,
                tc.tile_pool(name="pss", bufs=2, space="PSUM") as pss,
            ):
                for jt in range(NJT):
                    pv = psv.tile([128, D], F32, tag="pv")
                    for dt in range(4):
                        nc.tensor.matmul(
                            pv[:],
                            lhsT=xTb_t[dt][:, jt * 128:(jt + 1) * 128],
                            rhs=Wvb_t[dt][:],
                            start=(dt == 0),
                            stop=(dt == 3),
                        )
                    vt = vpool.tile([128, D], BF16, tag=f"V{jt}")
                    nc.scalar.copy(vt[:], pv[:])
                    V_t.append(vt)

                # sigma^T rows: [8, i]
                sig_row = sgpool.tile([H, HALF], F32, tag="sig_row")
                for ch in range(2):
                    ps = pss.tile([H, 512], F32, tag="ps")
                    for dt in range(4):
                        nc.tensor.matmul(
                            ps[:],
                            lhsT=Wsb_t[dt][:],
                            rhs=xTb_t[dt][:, PAD + ch * 512:PAD + (ch + 1) * 512],
                            start=(dt == 0),
                            stop=(dt == 3),
                        )
                    nc.scalar.activation(
                        sig_row[:, ch * 512:(ch + 1) * 512], ps[:], SIGM,
                        bias=bsig_t[:, 0:1],
                    )

            # r = sigmoid(-sigma); closed-form 1/den
            r_row = sgpool.tile([H, HALF], F32, tag="r_row")
            nc.scalar.activation(r_row[:], sig_row[:], SIGM, scale=-1.0)
            negr = sgpool.tile([H, HALF], F32, tag="negr")
            nc.vector.tensor_scalar_mul(negr[:], r_row[:], -1.0)
            z = sgpool.tile([H, HALF], F32, tag="z")
            nc.scalar.activation(z[:], negr[:], EXP)
            argA = sgpool.tile([H, HALF], F32, tag="argA")
            nc.vector.tensor_mul(argA[:], negr[:], ivp1_t[:])
            expA = sgpool.tile([H, HALF], F32, tag="expA")
            nc.scalar.activation(expA[:], argA[:], EXP)
            argB = sgpool.tile([H, HALF], F32, tag="argB")
            nc.vector.tensor_mul(argB[:], negr[:], ivnm_t[:])
            expB = sgpool.tile([H, HALF], F32, tag="expB")
            nc.scalar.activation(expB[:], argB[:], EXP)
            w = sgpool.tile([H, HALF], F32, tag="w")
            nc.vector.tensor_scalar(w[:], z[:], -1.0, 1.0, MUL, ADD)
            rw = sgpool.tile([H, HALF], F32, tag="rw")
            nc.vector.reciprocal(rw[:], w[:])
            t1 = sgpool.tile([H, HALF], F32, tag="t1")
            nc.vector.tensor_scalar_mul(t1[:], z[:], 2.0)
            nc.vector.tensor_sub(t1[:], t1[:], expA[:])
            nc.vector.tensor_sub(t1[:], t1[:], expB[:])
            t2 = sgpool.tile([H, HALF], F32, tag="t2")
            nc.vector.tensor_mul(t2[:], t1[:], rw[:])
            den = sgpool.tile([H, HALF], F32, tag="den")
            nc.vector.tensor_scalar_add(den[:], t2[:], 1.0)
            inv_row = sgpool.tile([H, HALF], F32, tag="inv_row")
            nc.vector.reciprocal(inv_row[:], den[:])

            # DMA partition-broadcast: R_all[h] = -r row -> 128 partitions,
            # Iv_all[h] = 1/den row -> 64 partitions.
            nc.sync.dma_start(negr_d[:, :], negr[:])
            nc.sync.dma_start(inv_d[:, :], inv_row[:])
            R_all, Iv_all = [], []
            for h in range(H):
                rt = bcpool.tile([128, HALF], F32, tag=f"R{h}")
                nc.sync.dma_start(
                    rt[:], negr_d[h:h + 1, :].to_broadcast((128, HALF))
                )
                R_all.append(rt)
                it = bcpool.tile([64, HALF], F32, tag=f"Iv{h}")
                nc.sync.dma_start(
                    it[:], inv_d[h:h + 1, :].to_broadcast((64, HALF))
                )
                Iv_all.append(it)

            # persistent out^T tiles (bf16): tile t = heads 2t,2t+1
            outT_t = []
            for t in range(4):
                oT = otpool.tile([128, HALF], BF16, tag=f"oT{t}")
                outT_t.append(oT)

            # ---------------- main loop ----------------
            with (
                tc.tile_pool(name="ap", bufs=3) as apool,
                tc.tile_pool(name="qp", bufs=3) as qpool,
                tc.tile_pool(name="fin", bufs=3) as fpool,
                tc.tile_pool(name="psa", bufs=4, space="PSUM") as psa,
                tc.tile_pool(name="psf", bufs=2, space="PSUM") as psf,
            ):
                for b in range(NBLK):
                    cols = slice(b * 128, (b + 1) * 128)
                    for h in range(H):
                        ARG = apool.tile([128, 384], F32, tag="ARG")
                        for o in range(3):
                            nc.vector.tensor_mul(
                                ARG[:, o * 128:(o + 1) * 128],
                                m2r_t[:, o * 128:(o + 1) * 128],
                                R_all[h][:, cols],
                            )
                        Q = qpool.tile([128, 384], BF16, tag="Q")
                        nc.scalar.activation(Q[:], ARG[:], EXP)
                        pav = psa.tile([64, 128], F32, tag="pav")
                        for o in range(3):
                            nc.tensor.matmul(
                                pav[:],
                                lhsT=V_t[b + o][:, h * 64:(h + 1) * 64],
                                rhs=Q[:, o * 128:(o + 1) * 128],
                                start=(o == 0),
                                stop=(o == 2),
                            )
                        nc.vector.tensor_mul(
                            outT_t[h // 2][
                                (h % 2) * 64:(h % 2) * 64 + 64, cols
                            ],
                            pav[:],
                            Iv_all[h][:, cols],
                        )
                    # ---- projection for i-block b ----
                    pf = psf.tile([128, D], F32, tag="pf")
                    for t in range(4):
                        nc.tensor.matmul(
                            pf[:],
                            lhsT=outT_t[t][:, cols],
                            rhs=Wob_t[t][:],
                            start=(t == 0),
                            stop=(t == 3),
                        )
                    fin = fpool.tile([128, D], F32, tag="fin")
                    nc.vector.tensor_add(fin[:], pf[:], bout_t[:])
                    nc.sync.dma_start(out[cols, :], fin[:])

    nc.compile()
    return nc


def _make_in_maps(x, W_v, W_sigma, b_sigma, W_out, b_out):
    m2r = np.empty((128, 384), dtype=np.float32)
    p = np.arange(128, dtype=np.float32)[:, None]
    q = np.arange(128, dtype=np.float32)[None, :]
    for o in range(3):
        m2r[:, o * 128:(o + 1) * 128] = np.abs(q - p + (1 - o) * 128.0)

    bf = ml_dtypes.bfloat16
    Wvb = np.ascontiguousarray(W_v.astype(bf))
    Wsb = np.ascontiguousarray(W_sigma.astype(bf))
    Wob = np.ascontiguousarray(W_out.astype(bf))
    bsig_c = np.ascontiguousarray(b_sigma.reshape(H, 1).astype(np.float32))
    bout_b = np.broadcast_to(b_out[None, :], (128, D)).copy().astype(np.float32)

    in_maps = []
    for c in range(8):
        bb, half = c // 2, c % 2
        i_start = half * HALF
        xp = np.zeros((NJROWS, D), dtype=np.float32)
        j_lo = max(0, i_start - PAD)
        j_hi = min(N, i_start + HALF + PAD)
        xp[j_lo - (i_start - PAD):j_hi - (i_start - PAD)] = x[bb, j_lo:j_hi]
        xTb = np.ascontiguousarray(xp.T.astype(bf))

        i_abs = (i_start + np.arange(HALF, dtype=np.float32))[None, :]
        ivp1 = np.broadcast_to(i_abs + 1.0, (H, HALF)).copy().astype(np.float32)
        ivnm = np.broadcast_to(float(N) - i_abs, (H, HALF)).copy().astype(np.float32)

        in_maps.append(
            {
                "xTb": xTb,
                "Wvb": Wvb,
                "Wsb": Wsb,
                "Wob": Wob,
                "bsig": bsig_c,
                "bout": bout_b,
                "m2r": m2r,
                "ivp1": ivp1,
                "ivnm": ivnm,
            }
        )
    return in_maps


def kernel(x, W_v, W_sigma, b_sigma, W_out, b_out):
    global _nc_cache
    x = np.asarray(x, dtype=np.float32)
    W_v = np.asarray(W_v, dtype=np.float32)
    W_sigma = np.asarray(W_sigma, dtype=np.float32)
    b_sigma = np.asarray(b_sigma, dtype=np.float32)
    W_out = np.asarray(W_out, dtype=np.float32)
    b_out = np.asarray(b_out, dtype=np.float32)

    if _nc_cache is None:
        _nc_cache = _build_nc()
    nc = _nc_cache

    in_maps = _make_in_maps(x, W_v, W_sigma, b_sigma, W_out, b_out)
    res = run_bass_kernel_spmd(nc, in_maps, core_ids=list(range(8)))

    out = np.empty((B, N, D), dtype=np.float32)
    for c in range(8):
        bb, half = c // 2, c % 2
        out[bb, half * HALF:(half + 1) * HALF, :] = res.results[c]["out"]
    return out
